# revision 49
# baseline (speedup 1.0000x reference)
"""Trainium2 Bass kernel for nn_MultiHeadAttention_61701500175237.

Sharding: 8 cores = 2 batches x 4 head-groups (4 heads each).
Each core computes Q/K/V projections for its (batch, 4-head) slice, RoPE,
causal attention, and a partial o_proj covering the full d_model; the host
sums the 4 partials per batch (the "all-reduce" of the hint, done at gather
time since the partials are independent and the harness gathers on host).

v2 schedule: projections are software-pipelined INTO the attention windows.
The softmax exp stream saturates the Scalar (ACT) engine during attention,
so every non-exp op is kept off ACT and the PE-idle slack under the exp
stream is filled with the next token-tile's QKV projection matmuls (and,
in the last window, the o_proj matmuls) instead of dummy HAM-warming fill:

  proj(0) | qt0 + proj(1) | qt1 + proj(2) | qt2 + proj(3) | qt3 + o_proj(0..2)
  | o_proj(3)

Device dataflow (per core, transposed-attention layout, bf16 matmul
operands with fp32 PSUM accumulation):
  - host passes x[b].T  -> xT [1024, 2048] bf16 (d on partitions: no
    on-device transposes anywhere)
  - QT/KT [j, tok] = W-shard.T (stationary) @ xT (moving)
  - RoPE in [j, tok] layout: weight rows are host-permuted per head to
    [evens 0:16 | odds 0:16 | evens 16:32 | odds 16:32] so the rotation
    partner lives 16 partitions away within the same 32-partition quadrant
    -> one DVE stream_shuffle provides the "swapped" operand; cos/sin are
    host tables.  All RoPE element-wise work runs on DVE (ACT is reserved
    for the exp stream).
  - logits^T [k, q] = KT-slice (stationary, K=64) @ QT-slice; two heads run
    concurrently in PE row-groups 0:64 / 64:128 (auto tile_position from
    the APs' base partitions); causally dead columns are trimmed from the
    matmul/exp/PV free ranges, the diagonal 128x128 block is masked by a
    0/1 multiply
  - P = exp(0.125 * logits^T) on ACT straight out of PSUM (the ONLY thing
    ACT ever does)
  - attn^T [d, q] (+ sumexp row) = [V | ones] (stationary) @ P; softmax
    denominator comes free as output row 64 of the same matmuls
  - 1/Z on DVE: the 1-partition sumexp row is DMA-reshaped to [128, 8],
    reciprocal'd on DVE at full lane parallelism, DMA'd back, then
    partition-broadcast on GpSimd and multiplied on DVE
  - o_proj: out[tok, n] = attn^T chunk (stationary) @ Wo-shard.T (moving),
    fp32 psum -> bf16 partial written to DRAM
"""

import sys

if "/opt/trn_rl_repo" not in sys.path:
    sys.path.insert(0, "/opt/trn_rl_repo")

import numpy as np
import ml_dtypes

import concourse.bass as bass  # noqa: F401
import concourse.tile as tile
from concourse import bacc, mybir

F32 = mybir.dt.float32
BF16 = mybir.dt.bfloat16
AF = mybir.ActivationFunctionType
NPBF16 = np.dtype(ml_dtypes.bfloat16)

B = 2
S = 2048
D_MODEL = 1024
N_HEADS = 16
D_K = 64
THETA = 10000.0

H_PER = 4          # heads per core
JW = H_PER * D_K   # 256: per-core projection width
N_CORES = 8
VSTRIDE = D_K + 1  # V tile col stride per head (64 data + 1 ones)
NDC = D_MODEL // 128  # 8 d-chunks
NT = S // 512      # 4 token-tiles / q-tiles

SWAP_MASK = list(range(16, 32)) + list(range(16))  # exchange 16-halves


def _act(nc, out, in_, func, scale=1.0):
    """ACT activation: out = func(in_*scale)."""
    return nc.scalar.activation(out, in_, func, bias=0.0, scale=float(scale))


_tables_pinned = False


def _pin_act_table():
    """Make every ACT func we emit resolve to one table so the kernel loads
    one table once (1.28us per reload avoided)."""
    global _tables_pinned
    if _tables_pinned:
        return
    _tables_pinned = True
    import concourse.bacc as bacc_mod

    orig = bacc_mod.get_activation_tables
    keep = "natural_log_exp_and_others"
    ours = {AF.Exp, AF.Ln, AF.Copy, AF.Identity}

    def pinned(arch):
        t = orig(arch)
        return {
            name: (funcs if name == keep else funcs - ours)
            for name, funcs in t.items()
        }

    bacc_mod.get_activation_tables = pinned


def _build_program():
    _pin_act_table()
    nc = bacc.Bacc("TRN2", target_bir_lowering=False, debug=False)

    xT = nc.dram_tensor("xT", [D_MODEL, S], BF16, kind="ExternalInput")
    wq = nc.dram_tensor("wq", [D_MODEL, JW], BF16, kind="ExternalInput")
    wk = nc.dram_tensor("wk", [D_MODEL, JW], BF16, kind="ExternalInput")
    wv = nc.dram_tensor("wv", [D_MODEL, JW], BF16, kind="ExternalInput")
    wo = nc.dram_tensor("wo", [JW, D_MODEL], BF16, kind="ExternalInput")
    cost = nc.dram_tensor("cost", [128, S], BF16, kind="ExternalInput")
    sint = nc.dram_tensor("sint", [128, S], BF16, kind="ExternalInput")
    maskt = nc.dram_tensor("maskt", [128, 128], BF16, kind="ExternalInput")
    outp = nc.dram_tensor("outp", [S, D_MODEL], BF16, kind="ExternalOutput")

    with tile.TileContext(nc) as tc:
        _body(tc, xT, wq, wk, wv, wo, cost, sint, maskt, outp)
    nc.compile()
    return nc


def _body(tc, xT, wq, wk, wv, wo, cost, sint, maskt, outp):
    nc = tc.nc

    with (
        tc.tile_pool(name="const", bufs=1) as cpool,
        tc.tile_pool(name="big", bufs=1) as bpool,
        tc.tile_pool(name="xtp", bufs=16) as xtp,
        tc.tile_pool(name="ropep", bufs=3) as ropep,
        tc.tile_pool(name="pp", bufs=3) as pp,
        tc.tile_pool(name="np_", bufs=2) as npool,
        tc.tile_pool(name="op", bufs=3) as op,
        tc.tile_pool(name="ps", space="PSUM", bufs=1) as ps,
    ):
        # --- resident weights / tables -----------------------------------
        # DMA issue costs ~0.6us of the issuing engine's queue per dma_start,
        # so spread the startup loads over sync/scalar/vector/gpsimd.  The
        # scalar queue is safe for startup-only DMAs: the first exp is
        # emitted long after and queues behind them.
        wq_sb = cpool.tile([128, NDC, JW], BF16, name="wq_sb")
        wk_sb = cpool.tile([128, NDC, JW], BF16, name="wk_sb")
        wv_sb = cpool.tile([128, NDC, JW], BF16, name="wv_sb")
        wqr = wq.rearrange("(c p) j -> p c j", p=128)
        wkr = wk.rearrange("(c p) j -> p c j", p=128)
        wvr = wv.rearrange("(c p) j -> p c j", p=128)
        xts0 = []
        for dc in range(NDC):
            xt0 = xtp.tile([128, 512], BF16, name=f"xt_0_{dc}", tag="xt",
                           bufs=24)
            nc.sync.dma_start(xt0[:], xT[dc * 128:(dc + 1) * 128, 0:512])
            xts0.append(xt0)
            nc.scalar.dma_start(wq_sb[:, dc], wqr[:, dc])
            nc.gpsimd.dma_start(wk_sb[:, dc], wkr[:, dc])
        cs_sb = cpool.tile([128, 2, S], BF16, name="cs_sb")
        for half in range(2):
            hsl = slice(half * (S // 2), (half + 1) * (S // 2))
            nc.scalar.dma_start(cs_sb[:, 0, hsl], cost[:, hsl])
            nc.gpsimd.dma_start(cs_sb[:, 1, hsl], sint[:, hsl])
        mask_sb = cpool.tile([128, 128], BF16, name="mask_sb")
        nc.gpsimd.dma_start(mask_sb[:], maskt[:])
        # wv split across the sync and gpsimd tails so all 8 chunks land
        # before the V-projection chains need them
        for dc in range(NDC):
            (nc.sync if dc < 4 else nc.gpsimd).dma_start(
                wv_sb[:, dc], wvr[:, dc])
        wo_sb = cpool.tile([128, 2, D_MODEL], BF16, name="wo_sb")
        wor = wo.rearrange("(c p) n -> p c n", p=128)
        for hc in range(2):
            nc.scalar.dma_start(wo_sb[:, hc], wor[:, hc])

        # --- persistent activations, one tile per token-tile -------------
        qts = [bpool.tile([128, 2, 512], BF16, name=f"qts_{t}")
               for t in range(NT)]
        kts = [bpool.tile([128, 2, 512], BF16, name=f"kts_{t}")
               for t in range(NT)]
        vts = [bpool.tile([128, 4, H_PER * VSTRIDE], BF16, name=f"vts_{t}")
               for t in range(NT)]
        ats = [bpool.tile([128, 2, 512], BF16, name=f"ats_{t}")
               for t in range(NT)]

        # ones columns for the fused softmax denominator; vts[0] on DVE
        # (needed in ~8us), the rest on gpsimd behind its startup DMAs
        for h in range(H_PER):
            nc.vector.memset(vts[0][:, :, h * VSTRIDE + D_K], 1.0)
        for t in range(1, NT):
            for h in range(H_PER):
                nc.gpsimd.memset(vts[t][:, :, h * VSTRIDE + D_K], 1.0)

        # HAM warmup: dependency-free dummy matmuls span the initial DMA
        # loads so the first projection chains start at full PE clock.
        wsc = cpool.tile([128, 512], BF16, name="wsc")
        nc.vector.memset(wsc[:], 0.0)
        wps = ps.tile([128, 512], F32, name="wps", tag="aux", bufs=2)
        for _ in range(14):
            nc.tensor.matmul(wps[:], wsc[:, 0:128], wsc[:],
                             start=True, stop=True)

        # ---------------- projection pieces ------------------------------
        xTr = xT.rearrange("(c p) s -> p c s", p=128)

        def issue_xt(tt):
            # one merged DMA per token-tile: issued a full window ahead so
            # the transfer hides under the previous window, and it frees
            # ~4us of sync-queue issue time per window for the normalize
            # DMAs.  (Weights stay per-chunk: the first chains consume them
            # at chunk granularity.)  Sync queue only: DMA issues carry
            # waits that must not head-block a compute-carrying queue.
            xt_t = xtp.tile([128, NDC, 512], BF16, name=f"xt_{tt}",
                            tag="xtb", bufs=2)
            nc.sync.dma_start(xt_t[:], xTr[:, :, tt * 512:(tt + 1) * 512])
            return [xt_t[:, dc] for dc in range(NDC)]

        def qk_fill(wsb, jg, xts, nm):
            aux = ps.tile([128, 512], F32, name=f"qkf_{nm}", tag="aux",
                          bufs=2)
            for dc in range(NDC):
                nc.tensor.matmul(
                    aux[:],
                    wsb[:, dc, jg * 128:(jg + 1) * 128],
                    xts[dc],
                    start=(dc == 0), stop=(dc == NDC - 1),
                    skip_group_check=True,
                )
            return aux

        def qk_rope(aux, dst, tt, jg, nm, act_evict=False):
            # RoPE: dst = aux*cos + shuffle16(aux)*sin', with the value and
            # its shuffle packed in one tile so one strided mul covers both.
            # act_evict moves the psum eviction to ACT for windows where the
            # DVE is the binding engine and ACT has slack.
            tsl = slice(tt * 512, (tt + 1) * 512)
            eq = ropep.tile([128, 2, 512], BF16, name=f"eq_{nm}", tag="eq")
            if act_evict:
                nc.scalar.copy(eq[:, 0, :], aux[:])
            else:
                nc.vector.tensor_copy(eq[:, 0, :], aux[:])
            nc.vector.stream_shuffle(eq[:, 1, :], eq[:, 0, :], SWAP_MASK)
            t12 = ropep.tile([128, 2, 512], BF16, name=f"t12_{nm}", tag="t12")
            nc.vector.tensor_mul(t12[:], eq[:], cs_sb[:, :, tsl])
            nc.vector.tensor_add(dst[:, jg, :], t12[:, 0, :], t12[:, 1, :])

        def v_fill(st, xts, nm):
            aux = ps.tile([128, 512], F32, name=f"vf_{nm}", tag="aux",
                          bufs=2)
            for dc in range(NDC):
                nc.tensor.matmul(
                    aux[:, 0:JW],
                    xts[dc][:, st * 128:(st + 1) * 128],
                    wv_sb[:, dc, :],
                    start=(dc == 0), stop=(dc == NDC - 1),
                    skip_group_check=True,
                )
            return aux

        def v_evict(aux, tt, st, act_evict=False):
            # one strided copy fills all 4 heads' 64-col runs
            dst = (vts[tt][:, st, :]
                   .rearrange("p (h s) -> p h s", h=H_PER)[:, :, 0:D_K])
            src = aux[:, 0:JW].rearrange("p (h s) -> p h s", h=H_PER)
            if act_evict:
                nc.scalar.copy(dst, src)
            else:
                nc.vector.tensor_copy(dst, src)

        def proj_pieces(tt):
            """Closure list for token-tile tt's QKV projection; popped a few
            per attention kt-iteration so the PE soaks exp-stream slack.
            The x-tile DMAs are issued eagerly (here) so they overlap the
            whole preceding window; fill and evict of each chain are split
            into separate pieces so the psum eviction lands one pop later
            and never head-blocks the PE queue."""
            xts = issue_xt(tt) if tt > 0 else [t[:] for t in xts0]
            # proj(2) pops during the qt1 window, where DVE is the binding
            # engine and ACT still has slack: evict on ACT there
            act_evict = False
            pieces = []
            state = {}

            def mk_qk_fill(key, wsb, jg, nm):
                def go():
                    state[key] = qk_fill(wsb, jg, xts, nm)
                return go

            def mk_qk_rope(key, dst, jg, nm):
                def go():
                    qk_rope(state.pop(key), dst, tt, jg, nm, act_evict)
                return go

            def mk_v_fill(key, st, nm):
                def go():
                    state[key] = v_fill(st, xts, nm)
                return go

            def mk_v_evict(key, st):
                def go():
                    v_evict(state.pop(key), tt, st, act_evict)
                return go

            for jg in range(2):
                for wsb, dsts, pnm in ((wq_sb, qts, "q"), (wk_sb, kts, "k")):
                    nm = f"{pnm}{tt}{jg}"
                    pieces.append(mk_qk_fill(f"f{nm}", wsb, jg, nm))
                    pieces.append(mk_qk_rope(f"f{nm}", dsts[tt], jg, nm))
            for st in range(4):
                key = f"v{tt}{st}"
                pieces.append(mk_v_fill(key, st, f"{tt}_{st}"))
                pieces.append(mk_v_evict(key, st))
            return pieces

        # ---------------- attention pieces -------------------------------
        def logits_pair(hp, qt, kt, psl_slot, c0):
            # both heads' logits; heads run in PE row-groups 0:64 / 64:128
            for hh in range(2):
                rows = slice(hh * 64, hh * 64 + 64)
                nc.tensor.matmul(
                    psl_slot[:, hh, c0:],
                    kts[kt // 4][rows, hp, (kt % 4) * 128:(kt % 4 + 1) * 128],
                    qts[qt][rows, hp, c0:],
                    start=True, stop=True,
                )

        def pv_pair(hp, qt, kt, p_slot, c0, pat, nkt):
            for hh in range(2):
                h = 2 * hp + hh
                nc.tensor.matmul(
                    pat[:, hh, c0:],
                    vts[kt // 4][:, kt % 4, h * VSTRIDE:h * VSTRIDE + VSTRIDE],
                    p_slot[:, hh, c0:],
                    start=(kt == 0), stop=(kt == nkt - 1),
                    skip_group_check=True,
                )

        def attn_kt(hp, qt, kt, pat):
            nfull = 4 * qt
            r = kt - nfull
            c0 = 128 * r if r >= 0 else 0
            psl = ps.tile([128, 2, 512], F32, name=f"psl_{hp}_{qt}_{kt}",
                          tag="psl", bufs=2)
            p = pp.tile([128, 2, 512], BF16, name=f"p_{hp}_{qt}_{kt}",
                        tag="p")
            logits_pair(hp, qt, kt, psl, c0)
            if r >= 0:
                # one strided op covers both heads' live ranges
                _act(nc, p[:, :, c0:], psl[:, :, c0:], AF.Exp, scale=0.125)
                nc.vector.tensor_mul(
                    p[:, :, c0:c0 + 128],
                    p[:, :, c0:c0 + 128],
                    mask_sb[:, None, :].broadcast_to((128, 2, 128)),
                )
            else:
                _act(nc, p[:], psl[:], AF.Exp, scale=0.125)
            return p, c0

        def norm_a(hp, qt, pat, st, tail=False):
            # evict pat to SBUF (frees the PSUM slot) and kick the sumexp
            # row out to a [128, 8] reshape so the reciprocal runs at full
            # DVE lane parallelism.  At the kernel tail the psum slot is
            # never needed again: only the sumexp row is copied (on the
            # now-idle ACT) and the muls read the psum directly.
            if tail:
                rowc = npool.tile([1, 2, 512], F32, name=f"rowc_{hp}_{qt}",
                                  tag="rowc")
                nc.scalar.copy(rowc[:], pat[64:65, :, :])
                rt = npool.tile([128, 8], F32, name=f"rt_{hp}_{qt}", tag="rt")
                nc.sync.dma_start(rt[:], rowc[:])
                st["patc"] = pat
                st["rt"] = rt
                return
            patc = npool.tile([65, 2, 512], F32, name=f"patc_{hp}_{qt}",
                              tag="patc")
            nc.vector.tensor_copy(patc[:], pat[:])
            rt = npool.tile([128, 8], F32, name=f"rt_{hp}_{qt}", tag="rt")
            nc.sync.dma_start(rt[:], patc[64:65, :, :])
            st["patc"] = patc
            st["rt"] = rt

        def norm_b1(hp, qt, st):
            # reciprocal + broadcast; split from the muls so the DVE FIFO
            # never sits head-blocked on the DMA/broadcast latency
            rt = st.pop("rt")
            rr = npool.tile([128, 8], F32, name=f"rr_{hp}_{qt}", tag="rr")
            nc.vector.reciprocal(rr[:], rt[:])
            r0 = npool.tile([1, 2, 512], F32, name=f"r0_{hp}_{qt}", tag="r0")
            nc.sync.dma_start(r0[:], rr[:])
            rb = npool.tile([64, 2, 512], F32, name=f"rb_{hp}_{qt}", tag="rb")
            nc.gpsimd.partition_broadcast(rb[:], r0[:])
            st["rb"] = rb

        def norm_b2(hp, qt, st):
            patc = st.pop("patc")
            rb = st.pop("rb")
            nc.vector.tensor_mul(
                ats[qt][0:64, hp, :], patc[0:64, 0, :], rb[:, 0, :]
            )
            tmp = npool.tile([64, 512], BF16, name=f"att_{hp}_{qt}", tag="att")
            nc.vector.tensor_mul(tmp[:], patc[0:64, 1, :], rb[:, 1, :])
            nc.sync.dma_start(ats[qt][64:128, hp, :], tmp[:])

        # ---------------- o_proj pieces ----------------------------------
        def oproj_tb(qt, tb, tail=False):
            # tail pieces alternate psum evictions between the (then-idle)
            # ACT and the DVE so the two aux slots drain in parallel
            rsl = slice(tb * 128, (tb + 1) * 128)
            gsl = slice(qt * 512 + tb * 128, qt * 512 + (tb + 1) * 128)
            oev = op.tile([128, D_MODEL], BF16, name=f"oev_{qt}_{tb}",
                          tag="oev")
            for nd in range(2):
                pso = ps.tile([128, 512], F32, name=f"pso_{qt}_{tb}_{nd}",
                              tag="aux", bufs=2)
                for hc in range(2):
                    nc.tensor.matmul(
                        pso[:],
                        ats[qt][:, hc, rsl],
                        wo_sb[:, hc, nd * 512:(nd + 1) * 512],
                        start=(hc == 0), stop=(hc == 1),
                        skip_group_check=True,
                    )
                if tail and nd == 0:
                    nc.scalar.copy(oev[:, nd * 512:(nd + 1) * 512], pso[:])
                else:
                    nc.vector.tensor_copy(oev[:, nd * 512:(nd + 1) * 512],
                                          pso[:])
            (nc.sync if tb % 2 == 0 else nc.gpsimd).dma_start(
                outp[gsl, :], oev[:])

        # two-pass o_proj for the last q-tile: the hc=0 contributions only
        # need hp0's normalize (done mid-window), so they run under the
        # hp1 exp stream; hc=1 lands after the tail normalize, merged on DVE
        oev3 = {}

        def oproj3_p1(tb):
            rsl = slice(tb * 128, (tb + 1) * 128)
            oev = op.tile([128, D_MODEL], F32, name=f"oev3_{tb}",
                          tag="oev3", bufs=4)
            oev3[tb] = oev
            for nd in range(2):
                pso = ps.tile([128, 512], F32, name=f"pso3_{tb}_{nd}",
                              tag="aux", bufs=2)
                nc.tensor.matmul(
                    pso[:],
                    ats[NT - 1][:, 0, rsl],
                    wo_sb[:, 0, nd * 512:(nd + 1) * 512],
                    start=True, stop=True, skip_group_check=True,
                )
                nc.vector.tensor_copy(oev[:, nd * 512:(nd + 1) * 512],
                                      pso[:])

        def oproj3_p2(tb):
            rsl = slice(tb * 128, (tb + 1) * 128)
            gsl = slice((NT - 1) * 512 + tb * 128,
                        (NT - 1) * 512 + (tb + 1) * 128)
            oev = oev3[tb]
            oevb = op.tile([128, D_MODEL], BF16, name=f"oevb_{tb}",
                           tag="oevb", bufs=4)
            for nd in range(2):
                pso = ps.tile([128, 512], F32, name=f"pso3b_{tb}_{nd}",
                              tag="aux", bufs=2)
                nc.tensor.matmul(
                    pso[:],
                    ats[NT - 1][:, 1, rsl],
                    wo_sb[:, 1, nd * 512:(nd + 1) * 512],
                    start=True, stop=True, skip_group_check=True,
                )
                nc.vector.tensor_add(oevb[:, nd * 512:(nd + 1) * 512],
                                     oev[:, nd * 512:(nd + 1) * 512],
                                     pso[:])
            (nc.sync if tb % 2 == 0 else nc.gpsimd).dma_start(
                outp[gsl, :], oevb[:])

        # ---------------- main schedule ----------------------------------
        from collections import deque
        pending = deque()
        # proj(0) runs up front (nothing else to overlap with); proj(1) is
        # staged first so its x-tile DMAs are issued before proj(0)'s
        # matmuls occupy the timeline
        pieces0 = proj_pieces(0)
        # defer tile-0's jg1 chains (pieces 4..7; only needed by hp1) into
        # the qt0 pop stream so attention starts ~4 chains earlier
        pending.extend(pieces0[4:8])
        pending.extend(proj_pieces(1))
        for piece in pieces0[0:4] + pieces0[8:]:
            piece()

        stash = {}
        for qt in range(NT):
            nkt = 4 * qt + 4
            # stage the work to interleave under this q-tile's exp stream:
            # the next token-tile's projection, or (last window) the o_proj
            # for all earlier q-tiles
            if 0 < qt < NT - 1:
                pending.extend(proj_pieces(qt + 1))
            elif qt == NT - 1:
                for oqt in range(NT - 1):
                    for tb in range(4):
                        pending.append(lambda q=oqt, t=tb: oproj_tb(q, t))
                # pass-1 of the last q-tile's o_proj comes last in the pop
                # order: by then hp0's normalize (hp1-kt4) has landed
                for tb in range(4):
                    pending.append(lambda t=tb: oproj3_p1(t))
            nstage = len(pending)
            total_kt = 2 * nkt
            # last window: front-load the pops so the tail is only the last
            # head-pair's normalize + o_proj merge pass
            spread = total_kt if qt < NT - 1 else 3 * total_kt
            popped = 0
            kt_i = 0
            for hp in range(2):
                prev = (1, qt - 1) if hp == 0 else (0, qt)
                pat = ps.tile([65, 2, 512], F32, name=f"pat_{hp}_{qt}",
                              tag="pat", bufs=1)
                for kt in range(nkt):
                    p, c0 = attn_kt(hp, qt, kt, pat)
                    # pops land between logits and pv so the PE has
                    # independent work while ACT streams the exp
                    kt_i += 1
                    want = min(nstage, (kt_i * nstage + spread - 1) // spread)
                    while popped < want and pending:
                        pending.popleft()()
                        popped += 1
                    if kt == 1 and prev in stash:
                        norm_b1(*prev, stash[prev])
                    if kt == min(4, nkt - 1) and prev in stash:
                        norm_b2(*prev, stash.pop(prev))
                    pv_pair(hp, qt, kt, p, c0, pat, nkt)
                nst = {}
                norm_a(hp, qt, pat, nst, tail=(qt == NT - 1 and hp == 1))
                stash[(hp, qt)] = nst
        # tail: the reserved o_proj pieces run while the last head-pair's
        # normalize chain (rowc -> rt -> recip -> r0 -> bcast -> muls)
        # drains its cross-engine latency
        def pop_n(n):
            for _ in range(n):
                if pending:
                    pending.popleft()()
        last = (1, NT - 1)
        pop_n(2)
        norm_b1(*last, stash[last])
        # drain ALL remaining pieces before the b2 muls: their DVE
        # evictions must precede the (broadcast-blocked) muls in the DVE
        # FIFO, or the aux rotation head-blocks and the PE stalls
        while pending:
            pending.popleft()()
        norm_b2(*last, stash.pop(last))
        for tb in range(4):
            oproj3_p2(tb)


# ---------------------------------------------------------------------------
# host-side sharding / tables
# ---------------------------------------------------------------------------

def _head_perm_and_freq():
    """Within-head row order [e0..e15 | o0..o15 | e16..e31 | o16..o31]
    (e_i = dim 2i, o_i = dim 2i+1) so the rope partner is 16 partitions away
    inside one 32-partition quadrant. Returns (perm, freq_idx, sin_sign)."""
    e = np.arange(0, D_K, 2)   # evens: x1, freq i = 0..31
    o = np.arange(1, D_K, 2)   # odds:  x2
    perm = np.concatenate([e[:16], o[:16], e[16:], o[16:]])
    freq = np.concatenate([np.arange(16), np.arange(16),
                           np.arange(16, 32), np.arange(16, 32)])
    sign = np.concatenate([-np.ones(16), np.ones(16),
                           -np.ones(16), np.ones(16)])
    return perm, freq, sign


def _rope_tables():
    half = D_K // 2
    inv_freq = THETA ** (-np.arange(half, dtype=np.float64) * 2.0 / D_K)
    ang = np.arange(S, dtype=np.float64)[None, :] * inv_freq[:, None]  # [32, S]
    cos32 = np.cos(ang)
    sin32 = np.sin(ang)
    _, freq, sign = _head_perm_and_freq()
    cos64 = cos32[freq]                      # [64, S]
    sin64 = sin32[freq] * sign[:, None]      # [64, S]
    cos128 = np.tile(cos64, (2, 1)).astype(NPBF16)
    sin128 = np.tile(sin64, (2, 1)).astype(NPBF16)
    return cos128, sin128


def _mask_table():
    kl = np.arange(128)[:, None]
    ql = np.arange(128)[None, :]
    return np.ascontiguousarray((ql >= kl).astype(NPBF16))


_nc_cache = None


def _get_nc():
    global _nc_cache
    if _nc_cache is None:
        _nc_cache = _build_program()
    return _nc_cache


def make_in_maps(x, Wq, Wk, Wv, Wo):
    x = np.asarray(x, dtype=np.float32)
    Wq = np.asarray(Wq, dtype=np.float32)
    Wk = np.asarray(Wk, dtype=np.float32)
    Wv = np.asarray(Wv, dtype=np.float32)
    Wo = np.asarray(Wo, dtype=np.float32)

    cos128, sin128 = _rope_tables()
    mask = _mask_table()
    perm, _, _ = _head_perm_and_freq()

    in_maps = []
    for c in range(N_CORES):
        b = c // 4
        hg = c % 4
        heads = np.arange(hg * H_PER, (hg + 1) * H_PER)
        rows_plain = (heads[:, None] * D_K + np.arange(D_K)[None, :]).reshape(-1)
        rows_perm = (heads[:, None] * D_K + perm[None, :]).reshape(-1)
        in_maps.append({
            "xT": np.ascontiguousarray(x[b].T).astype(NPBF16),
            "wq": np.ascontiguousarray(Wq[rows_perm, :].T).astype(NPBF16),
            "wk": np.ascontiguousarray(Wk[rows_perm, :].T).astype(NPBF16),
            "wv": np.ascontiguousarray(Wv[rows_plain, :].T).astype(NPBF16),
            "wo": np.ascontiguousarray(Wo[:, rows_plain].T).astype(NPBF16),
            "cost": cos128,
            "sint": sin128,
            "maskt": mask,
        })
    return in_maps


def gather_output(results):
    outs = [np.asarray(r["outp"], dtype=np.float32) for r in results]
    out = np.stack([
        outs[0] + outs[1] + outs[2] + outs[3],
        outs[4] + outs[5] + outs[6] + outs[7],
    ])
    return out.reshape(B, S, D_MODEL)


def _install_ntff_hook():
    """Provide antenv.axon_hooks + register the ctypes NTFF profile hook.

    The agent image's antenv package lacks axon_hooks, so trace=True under
    axon crashes on import. Recreate the tiny get/set module and drive
    profiling via direct ctypes calls into libaxon_pjrt.so (same ABI as
    trn_boot._ntff_profile_via_ctypes)."""
    import types
    import ctypes
    import contextlib

    if "antenv.axon_hooks" not in sys.modules:
        mod = types.ModuleType("antenv.axon_hooks")
        mod._hook = None

        def set_axon_ntff_profile_hook(h):
            mod._hook = h

        def get_axon_ntff_profile_hook():
            return mod._hook

        mod.set_axon_ntff_profile_hook = set_axon_ntff_profile_hook
        mod.get_axon_ntff_profile_hook = get_axon_ntff_profile_hook
        sys.modules["antenv.axon_hooks"] = mod
        import antenv

        antenv.axon_hooks = mod

    hooks = sys.modules["antenv.axon_hooks"]
    if hooks.get_axon_ntff_profile_hook() is not None:
        return

    so_path = "/opt/axon/libaxon_pjrt.so"
    try:
        lib = ctypes.CDLL(so_path)
    except OSError:
        return
    if not hasattr(lib, "axon_start_nrt_profile"):
        return
    lib.axon_start_nrt_profile.argtypes = [
        ctypes.POINTER(ctypes.c_int64), ctypes.c_size_t,
    ]
    lib.axon_start_nrt_profile.restype = ctypes.c_int64
    lib.axon_stop_nrt_profile.argtypes = [ctypes.c_char_p]
    lib.axon_stop_nrt_profile.restype = ctypes.c_int64

    @contextlib.contextmanager
    def _hook(output_dir, device_ids):
        import jax

        jax.devices()
        if device_ids:
            ids = (ctypes.c_int64 * len(device_ids))(*device_ids)
            rc = lib.axon_start_nrt_profile(ids, len(device_ids))
        else:
            rc = lib.axon_start_nrt_profile(None, 0)
        if rc != 0:
            raise RuntimeError(f"axon_start_nrt_profile rc={rc}")
        try:
            yield
        finally:
            n = lib.axon_stop_nrt_profile(str(output_dir).encode())
            print(f"profile: {n} file(s) written to {output_dir}")

    hooks.set_axon_ntff_profile_hook(_hook)


def kernel(x, Wq, Wk, Wv, Wo, _trace=False, _trace_cores=None):
    from concourse.bass_utils import run_bass_kernel_spmd

    if _trace:
        _install_ntff_hook()
    nc = _get_nc()
    in_maps = make_in_maps(x, Wq, Wk, Wv, Wo)
    res = run_bass_kernel_spmd(
        nc, in_maps, list(range(N_CORES)),
        trace=_trace, trace_cores=_trace_cores,
    )
    out = gather_output(res.results)
    if _trace:
        kernel.last_results = res
    return out
